# revision 65
# baseline (speedup 1.0000x reference)
"""Trainium2 Bass kernel for LinearSelfAttention3D (16x256x64x64, 8 heads, mem_kv).

Data-parallel over batch: 2 batches per core, 8 cores, identical SPMD program.
Per batch (x viewed [256, 4096] channel-major):
  Pass A (32 s-tiles of 128, ctx matmuls software-pipelined two tiles behind):
    kT = x^T @ w_k via ONE fp8-e4m3 DoubleRow matmul (contraction 256 in one
      instr at 0.5 cyc/row; weights pre-scaled x64 to clear fp8 subnormals,
      1/64 folded into the ACT exp scale)
    vT = x^T @ w_v at x64 scale via 3 fp8 DoubleRow matmuls with residual
      compensation (x8*wv8 + x8*wvr8 + xr8*wv8; plain fp8 v would cost ~3.7%
      error since v feeds ctx linearly - residuals cut that to ~0.1%); the
      x64 is descaled through w_out on the host
    expk = exp(kT/64) (ACT); vt staged [128,4,130] on DVE (Pool cannot access
      PSUM); ctx accumulated as 4 pair tiles [128,130], one PSUM bank each
      (interleaved accumulation groups must not share a bank - HW accumulation
      state is per-bank; cols 0:128 = pair v-dims, col 128 accumulates Z)
    mem_kv folded on host, added via identity matmul
    evac: rz=1/Z (DVE), cd diag blocks (ACT), cdt transpose (DVE),
      W_raw = cdt @ w_out' (PE), W_eff = W_raw * rz rows (DVE)
  Pass B (8 s-chunks of 512, stage_y drained at pipeline distance 3; before
    the final y drain, the NEXT batch's first 4 s-tiles are pre-staged on the
    freed k/v banks so DVE/ACT have vt/exp work during the drain):
    q = w_q @ x via fp8 DoubleRow, qps on a 4-bank rotation; expU (ACT)
    Zq via block-indicator fp16 matmul (PE); rb = 1/Zq (DVE recip approx)
    ep = expU*rb (1 on DVE, 3 on GPSIMD - SBUF-only ops are Pool-legal)
    y = W_eff^T @ ep over d (PE fp16), evac via ACT Identity, fp16 DMA out
Engine busy (TimelineSim): DVE 105 / ACT 103 / PE 94us, wall ~124us
(baseline was 197us predicted / 218us measured).
Measured end-to-end rel err ~5.1e-3 (gate 2e-2).
"""
import os
import sys

sys.path.insert(0, "/opt/trn_rl_repo")
import numpy as np
import ml_dtypes

import concourse.bass as bass  # noqa: E402
import concourse.bacc as bacc  # noqa: E402
import concourse.mybir as mybir  # noqa: E402
import concourse.tile as tile  # noqa: E402
from concourse import bass_utils  # noqa: E402

B, C, H, W = 16, 256, 64, 64
S = H * W  # 4096
MD, NH, HD, NM = 512, 8, 64, 4
SCALE = HD ** -0.5
EPS = 1e-5
N_CORES = 8
BPC = B // N_CORES
NCT = C // 128
NST = S // 128
NSC = S // 512
NDT = MD // 128
WS = 64.0  # fp8 weight prescale (keeps 0.02-scale weights out of subnormals)
F32 = mybir.dt.float32
F16 = mybir.dt.float16
F8 = mybir.dt.float8e4
NP8 = ml_dtypes.float8_e4m3
AF = mybir.ActivationFunctionType
DR = mybir.MatmulPerfMode.DoubleRow

_MODULE_CACHE = {}


def _build_module(has_bk, has_bv, has_bout):
    nc = bacc.Bacc(
        "TRN2",
        target_bir_lowering=False,
        debug=False,
        enable_asserts=False,
        num_devices=N_CORES,
    )
    x8_d = nc.dram_tensor("x8", (BPC, 128, 2, S), F8, kind="ExternalInput").ap()
    xr8_d = nc.dram_tensor("xr8", (BPC, 128, 2, S), F8, kind="ExternalInput").ap()
    w8k_d = nc.dram_tensor("w8k", (128, 2, MD), F8, kind="ExternalInput").ap()
    w8q_d = nc.dram_tensor("w8q", (128, NDT, 2, 128), F8, kind="ExternalInput").ap()
    wv8_d = nc.dram_tensor("wv8", (128, 2, MD), F8, kind="ExternalInput").ap()
    wvr8_d = nc.dram_tensor("wvr8", (128, 2, MD), F8, kind="ExternalInput").ap()
    woutT_d = nc.dram_tensor("woutT", (NDT, 128, C), F16, kind="ExternalInput").ap()
    bq_d = nc.dram_tensor("bq", (128, NDT), F32, kind="ExternalInput").ap()
    bones_d = nc.dram_tensor("bones", (128, 128), F16, kind="ExternalInput").ap()
    cmem_d = nc.dram_tensor("cmem", (128, NDT, 130), F16, kind="ExternalInput").ap()
    ident_d = nc.dram_tensor("ident", (128, 128), F16, kind="ExternalInput").ap()
    y_d = nc.dram_tensor("y", (BPC, 2, 128, S), F16, kind="ExternalOutput").ap()
    if has_bk or has_bv:
        onesrow_d = nc.dram_tensor("onesrow", (2, 128), F16, kind="ExternalInput").ap()
        bkv_d = nc.dram_tensor("bkv", (2, 2 * MD), F16, kind="ExternalInput").ap()
    if has_bout:
        bout_d = nc.dram_tensor("bout", (128, 2), F32, kind="ExternalInput").ap()

    with tile.TileContext(nc) as tc, nc.allow_low_precision(reason="fp8/fp16 matmul operands"):
        import contextlib

        cstack = contextlib.ExitStack()
        const = cstack.enter_context(tc.tile_pool(name="const", bufs=1))
        xrp = cstack.enter_context(tc.tile_pool(name="xrp", bufs=1))
        work = cstack.enter_context(tc.tile_pool(name="work", bufs=8))
        weffp = cstack.enter_context(tc.tile_pool(name="weffp", bufs=8))
        pool9 = cstack.enter_context(tc.tile_pool(name="pool9", bufs=17))

        def load_c(shape, dt, src_ap, tag, n_splits=None):
            t = const.tile(list(shape), dt, tag=tag, name=tag)
            if n_splits is None:
                nc.sync.dma_start(t[:], src_ap)
            else:
                for i in range(n_splits):
                    nc.sync.dma_start(t[:, i], src_ap[i])
            return t

        # DMA priority: k-path operands first so k matmuls start earliest,
        # then v-path, then the rest; batch-0 x split by s-range
        w8k_r = load_c((128, 2, MD), F8, w8k_d, "w8k")
        x8_t, xr8_t = [], []
        for b in range(BPC):
            x8_t.append(xrp.tile([128, 2, S], F8, tag=f"x8_{b}", name=f"x8_{b}"))
            xr8_t.append(xrp.tile([128, 2, S], F8, tag=f"xr8_{b}", name=f"xr8_{b}"))
        nc.sync.dma_start(x8_t[0][:, :, 0:S // 8], x8_d[0][:, :, 0:S // 8])
        wv8_r = load_c((128, 2, MD), F8, wv8_d, "wv8")
        wvr8_r = load_c((128, 2, MD), F8, wvr8_d, "wvr8")
        nc.sync.dma_start(xr8_t[0][:, :, 0:S // 8], xr8_d[0][:, :, 0:S // 8])
        nc.sync.dma_start(x8_t[0][:, :, S // 8:S // 4], x8_d[0][:, :, S // 8:S // 4])
        nc.sync.dma_start(xr8_t[0][:, :, S // 8:S // 4], xr8_d[0][:, :, S // 8:S // 4])
        w8q_r = load_c((128, NDT, 2, 128), F8, w8q_d, "w8q")
        for q in range(1, 4):
            sq = slice(q * (S // 4), (q + 1) * (S // 4))
            nc.sync.dma_start(x8_t[0][:, :, sq], x8_d[0][:, :, sq])
            nc.sync.dma_start(xr8_t[0][:, :, sq], xr8_d[0][:, :, sq])
        wo_r = load_c((128, NDT, C), F16, woutT_d, "wo", n_splits=NDT)
        bones_r = load_c((128, 128), F16, bones_d, "bones")
        cmem_r = load_c((128, NDT, 130), F16, cmem_d, "cmem")
        ident_r = load_c((128, 128), F16, ident_d, "ident")
        bq_t = const.tile([128, NDT], F32, tag="bq")
        nc.sync.dma_start(bq_t[:], bq_d)
        if has_bk or has_bv:
            onesrow_r = load_c((2, 128), F16, onesrow_d, "onesrow")
            bkv_r = load_c((2, 2 * MD), F16, bkv_d, "bkv")
        if has_bout:
            bout_t = const.tile([128, 2], F32, tag="bout")
            nc.sync.dma_start(bout_t[:], bout_d)
        for b in range(1, BPC):
            nc.sync.dma_start(x8_t[b][:], x8_d[b])
            nc.sync.dma_start(xr8_t[b][:], xr8_d[b])

        zero_r = const.tile([128, 128], F16, tag="zeror")
        nc.vector.memset(zero_r[:], 0.0)
        wzero = const.tile([128, 512], F16, tag="wzero")
        nc.vector.memset(wzero[:], 0.0)

        # persistent blockdiag buffers: off-diagonal zeros written once
        cds, cdts = [], []
        for p in range(NDT):
            cd = const.tile([128, 128], F16, tag=f"cd{p}", name=f"cd{p}")
            cdt = const.tile([128, 128], F16, tag=f"cdt{p}", name=f"cdt{p}")
            nc.vector.memset(cd[:], 0.0)
            nc.vector.memset(cdt[:], 0.0)
            cds.append(cd)
            cdts.append(cdt)
        rz4 = const.tile([128, NDT], F32, tag="rz4")
        # vt ring: ones cols (Z accumulators) written once, v-cols each s-tile
        vts = []
        for i in range(8):
            vt = const.tile([128, NDT, 130], F16, tag=f"vt{i}", name=f"vt{i}")
            nc.vector.memset(vt[:, :, 128:130], 1.0)
            vts.append(vt)

        # PE warmup: ramps p-state and covers the batch-0 input DMA
        with tc.tile_pool(name="warm", bufs=1, space="PSUM") as pw:
            wps = pw.tile([128, 512], F32, tag="warm")
            for i in range(2):
                nc.tensor.matmul(wps[:], zero_r[:], wzero[:],
                                 start=True, stop=True)

        def emit_tile(pp, bt, st):
            # one pass-A s-tile for batch bt: k + v matmuls, exp, vt evac
            x8b, xr8b = x8_t[bt], xr8_t[bt]
            sl = slice(st * 128, (st + 1) * 128)
            kps = pp.tile([128, 512], F32, tag=f"k{st % 2}",
                          name=f"k_{bt}_{st}")
            nc.tensor.matmul(kps[:], x8b[:, :, sl], w8k_r[:],
                             start=True, stop=(not has_bk), perf_mode=DR)
            if has_bk:
                nc.tensor.matmul(kps[:], onesrow_r[:], bkv_r[:, 0:MD],
                                 start=False, stop=True)
            # v at x64 scale via fp8 DoubleRow with residual compensation:
            # x8*wv8 + x8*wvr8 + xr8*wv8 (descaled through w_out on host)
            vps = pp.tile([128, 512], F32, tag=f"v{st % 2}",
                          name=f"v_{bt}_{st}")
            nc.tensor.matmul(vps[:], x8b[:, :, sl], wv8_r[:],
                             start=True, stop=False, perf_mode=DR)
            nc.tensor.matmul(vps[:], x8b[:, :, sl], wvr8_r[:],
                             start=False, stop=False, perf_mode=DR)
            nc.tensor.matmul(vps[:], xr8b[:, :, sl], wv8_r[:],
                             start=False, stop=(not has_bv), perf_mode=DR)
            if has_bv:
                nc.tensor.matmul(vps[:], onesrow_r[:], bkv_r[:, MD:2 * MD],
                                 start=False, stop=True)
            ek = work.tile([128, 512], F16, tag="ek", name=f"ek_{bt}_{st}")
            nc.scalar.activation(ek[:], kps[:], AF.Exp, scale=1.0 / WS)
            vt = vts[st % 8]
            # PSUM evac must stay off GPSIMD (no PSUM access on Pool)
            nc.vector.tensor_copy(
                vt[:, :, 0:128],
                vps[:].rearrange("p (g c) -> p g c", g=NDT),
            )
            return (ek, vt, st)

        NPRE = 4  # s-tiles of the next batch pre-staged before the y drain
        pre_staged = {}
        for b in range(BPC):
            x8_r, xr8_r = x8_t[b], xr8_t[b]
            with tc.tile_pool(name=f"pp{b}", bufs=1, space="PSUM") as pp:
                # one PSUM bank per ctx pair: interleaved accumulation groups
                # must NOT share a bank (HW accumulate state is per-bank)
                ctxps = [
                    pp.tile([128, 130], F32, tag=f"c{p}", name=f"ctx{p}_{b}")[:]
                    for p in range(NDT)
                ]

                # ---- pass A ----
                def ctx_mms(ek, vt, st):
                    for p in range(NDT):
                        nc.tensor.matmul(
                            ctxps[p][:],
                            ek[:, p * 128:(p + 1) * 128],
                            vt[:, p, :],
                            start=(st == 0),
                            stop=False,
                        )

                apend = list(pre_staged.pop(b, []))
                for st in range(len(apend), NST):
                    entry = emit_tile(pp, b, st)
                    while len(apend) > 1:
                        ctx_mms(*apend.pop(0))
                    apend.append(entry)
                for it in apend:
                    ctx_mms(*it)
                # ---- mem add + ctx evac, interleaved per pair ----
                for p in range(NDT):
                    nc.tensor.matmul(ctxps[p][:], ident_r[:], cmem_r[:, p, :],
                                     start=False, stop=True)
                    nc.vector.reciprocal_approx_fast(
                        rz4[:, p:p + 1], ctxps[p][:, 128:129]
                    )
                    nc.scalar.copy(cds[p][0:64, 0:64], ctxps[p][0:64, 0:64])
                    nc.scalar.copy(cds[p][64:128, 64:128],
                                   ctxps[p][64:128, 64:128])
                    for h in range(2):
                        o = 64 * h
                        for a in range(2):
                            for cc in range(2):
                                nc.vector.transpose(
                                    cdts[p][o + 32 * a:o + 32 * a + 32,
                                            o + 32 * cc:o + 32 * cc + 32],
                                    cds[p][o + 32 * cc:o + 32 * cc + 32,
                                           o + 32 * a:o + 32 * a + 32],
                                )
                weffs = []
                for p in range(NDT):
                    wraw = pp.tile([128, C], F32, tag=f"k{p % 2}",
                                   name=f"wraw{b}_{p}")
                    nc.tensor.matmul(wraw[:], cdts[p][:], wo_r[:, p, :],
                                     start=True, stop=True)
                    weff = weffp.tile([128, C], F16, tag="weff",
                                      name=f"weff_{b}_{p}")
                    nc.vector.tensor_scalar_mul(weff[:], wraw[:], rz4[:, p:p + 1])
                    weffs.append(weff)

                # ---- pass B ----
                def stage_y(sc, eps):
                    s0 = sc * 512
                    for ot in range(2):
                        yps = pp.tile([128, 512], F32, tag=f"c{ot}",
                                      name=f"y{b}_{sc}_{ot}")
                        for d in range(NDT):
                            nc.tensor.matmul(
                                yps[:],
                                weffs[d][:, ot * 128:(ot + 1) * 128],
                                eps[d][:],
                                start=(d == 0),
                                stop=(d == NDT - 1),
                            )
                        ysb = pool9.tile([128, 512], F16, tag="ysb")
                        if has_bout:
                            nc.scalar.activation(ysb[:], yps[:], AF.Identity,
                                                 bias=bout_t[:, ot:ot + 1])
                        else:
                            nc.scalar.activation(ysb[:], yps[:], AF.Identity)
                        nc.sync.dma_start(y_d[b, ot, :, s0:s0 + 512], ysb[:])

                pending = []
                depth = 3
                for sc in range(NSC):
                    # drain BEFORE this chunk's exps: ACT is in-order, so ysb
                    # must precede exps whose inputs aren't ready yet
                    if len(pending) > depth:
                        stage_y(*pending.pop(0))
                    s0 = sc * 512
                    eps = []
                    for d in range(NDT):
                        # 4-bank rotation decouples q matmuls from exp
                        # backlog; chunk 0 avoids k0/k1 so the wraw matmuls
                        # (same banks) run concurrently with the first exps
                        qtag = ("v0", "v1", "v0", "v1") if sc == 0 else \
                            ("v0", "v1", "k0", "k1")
                        qps = pp.tile([128, 512], F32, tag=qtag[d],
                                      name=f"q{b}_{sc}_{d}")
                        nc.tensor.matmul(qps[:], w8q_r[:, d],
                                         x8_r[:, :, s0:s0 + 512],
                                         start=True, stop=True,
                                         perf_mode=DR)
                        eu = pool9.tile([128, 512], F16, tag="eu")
                        nc.scalar.activation(eu[:], qps[:], AF.Exp,
                                             bias=bq_t[:, d:d + 1],
                                             scale=1.0 / WS)
                        zqb = pp.tile([128, 512], F32, tag=f"c{2 + d % 2}",
                                      name=f"zqb{b}_{sc}_{d}")
                        nc.tensor.matmul(zqb[:], bones_r[:], eu[:],
                                         start=True, stop=True)
                        rb = pool9.tile([128, 512], F32, tag="rb")
                        nc.vector.reciprocal_approx_fast(rb[:], zqb[:])
                        ep = pool9.tile([128, 512], F16, tag="ep")
                        # one normalize-mul on DVE, three on the idle GPSIMD
                        mul_eng = nc.vector if d == 0 else nc.gpsimd
                        mul_eng.tensor_mul(ep[:], eu[:], rb[:])
                        eps.append(ep)
                    pending.append((sc, eps))
                if b + 1 < BPC:
                    # pre-stage the next batch's first s-tiles before the y
                    # drain: k/v banks are free, and this feeds DVE/ACT work
                    # (vt, exp) into the drain where they would idle
                    pre_staged[b + 1] = [emit_tile(pp, b + 1, st)
                                        for st in range(NPRE)]
                for it in pending:
                    stage_y(*it)
        cstack.close()

    nc.compile()
    return nc


def _prep_consts(w_qkv, b_qkv, mem_kv, w_out, b_out, bn_gamma, bn_beta, bn_mean, bn_var):
    w_qkv = np.asarray(w_qkv, np.float32)
    b_qkv = np.asarray(b_qkv, np.float32)
    mem_kv = np.asarray(mem_kv, np.float32)
    w_out = np.asarray(w_out, np.float32)
    b_out = np.asarray(b_out, np.float32)
    g = np.asarray(bn_gamma, np.float64)
    be = np.asarray(bn_beta, np.float64)
    mu = np.asarray(bn_mean, np.float64)
    var = np.asarray(bn_var, np.float64)

    inv = g / np.sqrt(var + EPS)
    # SCALE (softmax(q) * HD**-0.5) is folded into the output projection
    w_out_f = (w_out.astype(np.float64) * inv[:, None] * SCALE).astype(np.float32)
    b_out_f = ((b_out.astype(np.float64) - mu) * inv + be).astype(np.float32)

    wq = w_qkv[0:MD]          # [512, 256]
    wk = w_qkv[MD:2 * MD]
    wv = w_qkv[2 * MD:]

    consts = {}
    # w8k[p, t, o] = wk[o, t*128+p] * WS
    consts["w8k"] = np.ascontiguousarray(
        (wk.T * WS).reshape(2, 128, MD).transpose(1, 0, 2)
    ).astype(NP8)
    # w8q[p, d, t, m] = wq[d*128+m, t*128+p] * WS
    w8q = (wq * WS).reshape(NDT, 128, 2, 128)  # [d, m, t, p]
    consts["w8q"] = np.ascontiguousarray(w8q.transpose(3, 0, 2, 1)).astype(NP8)
    # v weights at x64 scale + fp8 residual; the x64 is descaled via w_out
    wv64 = np.ascontiguousarray((wv.T * WS).reshape(2, 128, MD).transpose(1, 0, 2))
    wv8 = wv64.astype(NP8)
    consts["wv8"] = wv8
    consts["wvr8"] = (wv64 - wv8.astype(np.float32)).astype(NP8)
    consts["woutT"] = np.ascontiguousarray(
        (w_out_f / WS).T.reshape(NDT, 128, C)
    ).astype(np.float16)
    consts["bq"] = np.ascontiguousarray(b_qkv[0:MD].reshape(NDT, 128).T)

    bones = np.zeros((128, 128), np.float16)
    bones[0:64, 0:64] = 1.0
    bones[64:128, 64:128] = 1.0
    consts["bones"] = bones

    mk = mem_kv[0].astype(np.float64)
    mv = mem_kv[1].astype(np.float64)
    emk = np.exp(mk)
    ctx_mem = np.einsum("him,hjm->hij", emk, mv)
    z_mem = emk.sum(-1)
    cmem = np.zeros((128, NDT, 130), np.float16)
    for p in range(NDT):
        for t in range(2):
            h = 2 * p + t
            r0 = 64 * t
            # ctx rows are at the x64 v-scale; Z col stays unscaled
            cmem[r0:r0 + 64, p, r0:r0 + 64] = ctx_mem[h] * WS
            cmem[r0:r0 + 64, p, 128] = z_mem[h]
    consts["cmem"] = cmem
    consts["ident"] = np.eye(128, dtype=np.float16)

    has_bk = bool(np.any(b_qkv[MD:2 * MD] != 0))
    has_bv = bool(np.any(b_qkv[2 * MD:] != 0))
    has_bout = bool(np.any(b_out_f != 0))
    if has_bk or has_bv:
        # K=2 rank-2 form: ones row + zero row (k bias pre-scaled like w8k)
        onesrow = np.zeros((2, 128), np.float16)
        onesrow[0] = 1.0
        consts["onesrow"] = onesrow
        bkv = np.zeros((2, 2 * MD), np.float16)
        bkv[0, 0:MD] = (b_qkv[MD:2 * MD] * WS).astype(np.float16)
        bkv[0, MD:] = (b_qkv[2 * MD:] * WS).astype(np.float16)
        consts["bkv"] = bkv
    if has_bout:
        consts["bout"] = np.ascontiguousarray(b_out_f.reshape(2, 128).T)
    return consts, has_bk, has_bv, has_bout


def kernel(x, w_qkv, b_qkv, mem_kv, w_out, b_out, bn_gamma, bn_beta, bn_mean, bn_var):
    x = np.asarray(x, np.float32)
    consts, has_bk, has_bv, has_bout = _prep_consts(
        w_qkv, b_qkv, mem_kv, w_out, b_out, bn_gamma, bn_beta, bn_mean, bn_var
    )

    key = (has_bk, has_bv, has_bout)
    if key not in _MODULE_CACHE:
        _MODULE_CACHE[key] = _build_module(*key)
    nc = _MODULE_CACHE[key]

    # x8[b, p, t, s] = x[b, t*128+p, s]; xr8 = fp8 residual of the fp8 cast
    xt = x.reshape(B, 2, 128, S).transpose(0, 2, 1, 3)
    x8 = np.ascontiguousarray(xt).astype(NP8)
    xr8 = (xt - x8.astype(np.float32)).astype(NP8)
    in_maps = []
    for c in range(N_CORES):
        m = dict(consts)
        m["x8"] = np.ascontiguousarray(x8[c * BPC:(c + 1) * BPC])
        m["xr8"] = np.ascontiguousarray(xr8[c * BPC:(c + 1) * BPC])
        in_maps.append(m)

    trace = bool(int(os.environ.get("BASS_KERNEL_TRACE", "0")))
    res = bass_utils.run_bass_kernel_spmd(
        nc, in_maps, core_ids=list(range(N_CORES)), trace=trace
    )
    if trace:
        kernel.last_exec_time_ns = res.exec_time_ns
        kernel.last_mean_exec_time_ns = res.mean_exec_time_ns

    y = np.stack([res.results[c]["y"] for c in range(N_CORES)])
    y = y.reshape(B, C, H, W).astype(np.float32)
    return y


# revision 68
# speedup vs baseline: 1.0026x; 1.0026x over previous
"""Trainium2 Bass kernel for LinearSelfAttention3D (16x256x64x64, 8 heads, mem_kv).

Data-parallel over batch: 2 batches per core, 8 cores, identical SPMD program.
Per batch (x viewed [256, 4096] channel-major):
  Pass A (32 s-tiles of 128, ctx matmuls software-pipelined two tiles behind):
    kT = x^T @ w_k via ONE fp8-e4m3 DoubleRow matmul (contraction 256 in one
      instr at 0.5 cyc/row; weights pre-scaled x64 to clear fp8 subnormals,
      1/64 folded into the ACT exp scale)
    vT = x^T @ w_v at x64 scale via 3 fp8 DoubleRow matmuls with residual
      compensation (x8*wv8 + x8*wvr8 + xr8*wv8; plain fp8 v would cost ~3.7%
      error since v feeds ctx linearly - residuals cut that to ~0.1%); the
      x64 is descaled through w_out on the host
    expk = exp(kT/64) (ACT); vt staged [128,4,130] on DVE (Pool cannot access
      PSUM); ctx accumulated as 4 pair tiles [128,130], one PSUM bank each
      (interleaved accumulation groups must not share a bank - HW accumulation
      state is per-bank; cols 0:128 = pair v-dims, col 128 accumulates Z)
    mem_kv folded on host, added via identity matmul
    evac: rz=1/Z (DVE), cd diag blocks (ACT), cdt transpose (DVE),
      W_raw = cdt @ w_out' (PE), W_eff = W_raw * rz rows (DVE)
  Pass B (8 s-chunks of 512, stage_y drained at pipeline distance 3; before
    the final y drain, the NEXT batch's first 4 s-tiles are pre-staged on the
    freed k/v banks so DVE/ACT have vt/exp work during the drain):
    q = w_q @ x via fp8 DoubleRow, qps on a 4-bank rotation; expU (ACT)
    Zq via block-indicator fp16 matmul (PE); rb = 1/Zq (DVE recip approx)
    ep = expU*rb (1 on DVE, 3 on GPSIMD - SBUF-only ops are Pool-legal)
    y = W_eff^T @ ep over d (PE fp16), evac via ACT Identity, fp16 DMA out
Engine busy (TimelineSim): DVE 105 / ACT 103 / PE 94us, wall ~124us
(baseline was 197us predicted / 218us measured).
Measured end-to-end rel err ~5.1e-3 (gate 2e-2).
"""
import os
import sys

sys.path.insert(0, "/opt/trn_rl_repo")
import numpy as np
import ml_dtypes

import concourse.bass as bass  # noqa: E402
import concourse.bacc as bacc  # noqa: E402
import concourse.mybir as mybir  # noqa: E402
import concourse.tile as tile  # noqa: E402
from concourse import bass_utils  # noqa: E402

B, C, H, W = 16, 256, 64, 64
S = H * W  # 4096
MD, NH, HD, NM = 512, 8, 64, 4
SCALE = HD ** -0.5
EPS = 1e-5
N_CORES = 8
BPC = B // N_CORES
NCT = C // 128
NST = S // 128
NSC = S // 512
NDT = MD // 128
WS = 64.0  # fp8 weight prescale (keeps 0.02-scale weights out of subnormals)
F32 = mybir.dt.float32
F16 = mybir.dt.float16
F8 = mybir.dt.float8e4
NP8 = ml_dtypes.float8_e4m3
AF = mybir.ActivationFunctionType
DR = mybir.MatmulPerfMode.DoubleRow

_MODULE_CACHE = {}


def _build_module(has_bk, has_bv, has_bout):
    nc = bacc.Bacc(
        "TRN2",
        target_bir_lowering=False,
        debug=False,
        enable_asserts=False,
        num_devices=N_CORES,
    )
    x8_d = nc.dram_tensor("x8", (BPC, 128, 2, S), F8, kind="ExternalInput").ap()
    xr8_d = nc.dram_tensor("xr8", (BPC, 128, 2, S), F8, kind="ExternalInput").ap()
    w8k_d = nc.dram_tensor("w8k", (128, 2, MD), F8, kind="ExternalInput").ap()
    w8q_d = nc.dram_tensor("w8q", (128, NDT, 2, 128), F8, kind="ExternalInput").ap()
    wv8_d = nc.dram_tensor("wv8", (128, 2, MD), F8, kind="ExternalInput").ap()
    wvr8_d = nc.dram_tensor("wvr8", (128, 2, MD), F8, kind="ExternalInput").ap()
    woutT_d = nc.dram_tensor("woutT", (NDT, 128, C), F16, kind="ExternalInput").ap()
    bq_d = nc.dram_tensor("bq", (128, NDT), F32, kind="ExternalInput").ap()
    bones_d = nc.dram_tensor("bones", (128, 128), F16, kind="ExternalInput").ap()
    cmem_d = nc.dram_tensor("cmem", (128, NDT, 130), F16, kind="ExternalInput").ap()
    ident_d = nc.dram_tensor("ident", (128, 128), F16, kind="ExternalInput").ap()
    y_d = nc.dram_tensor("y", (BPC, 2, 128, S), F16, kind="ExternalOutput").ap()
    if has_bk or has_bv:
        onesrow_d = nc.dram_tensor("onesrow", (2, 128), F16, kind="ExternalInput").ap()
        bkv_d = nc.dram_tensor("bkv", (2, 2 * MD), F16, kind="ExternalInput").ap()
    if has_bout:
        bout_d = nc.dram_tensor("bout", (128, 2), F32, kind="ExternalInput").ap()

    with tile.TileContext(nc) as tc, nc.allow_low_precision(reason="fp8/fp16 matmul operands"):
        import contextlib

        cstack = contextlib.ExitStack()
        const = cstack.enter_context(tc.tile_pool(name="const", bufs=1))
        xrp = cstack.enter_context(tc.tile_pool(name="xrp", bufs=1))
        work = cstack.enter_context(tc.tile_pool(name="work", bufs=8))
        weffp = cstack.enter_context(tc.tile_pool(name="weffp", bufs=8))
        pool9 = cstack.enter_context(tc.tile_pool(name="pool9", bufs=17))

        def load_c(shape, dt, src_ap, tag, n_splits=None):
            t = const.tile(list(shape), dt, tag=tag, name=tag)
            if n_splits is None:
                nc.sync.dma_start(t[:], src_ap)
            else:
                for i in range(n_splits):
                    nc.sync.dma_start(t[:, i], src_ap[i])
            return t

        # DMA priority: k-path operands first so k matmuls start earliest,
        # then v-path, then the rest; batch-0 x split by s-range
        w8k_r = load_c((128, 2, MD), F8, w8k_d, "w8k")
        x8_t, xr8_t = [], []
        for b in range(BPC):
            x8_t.append(xrp.tile([128, 2, S], F8, tag=f"x8_{b}", name=f"x8_{b}"))
            xr8_t.append(xrp.tile([128, 2, S], F8, tag=f"xr8_{b}", name=f"xr8_{b}"))
        nc.sync.dma_start(x8_t[0][:, :, 0:S // 8], x8_d[0][:, :, 0:S // 8])
        wv8_r = load_c((128, 2, MD), F8, wv8_d, "wv8")
        wvr8_r = load_c((128, 2, MD), F8, wvr8_d, "wvr8")
        nc.sync.dma_start(xr8_t[0][:, :, 0:S // 8], xr8_d[0][:, :, 0:S // 8])
        nc.sync.dma_start(x8_t[0][:, :, S // 8:S // 4], x8_d[0][:, :, S // 8:S // 4])
        nc.sync.dma_start(xr8_t[0][:, :, S // 8:S // 4], xr8_d[0][:, :, S // 8:S // 4])
        w8q_r = load_c((128, NDT, 2, 128), F8, w8q_d, "w8q")
        for q in range(1, 4):
            sq = slice(q * (S // 4), (q + 1) * (S // 4))
            nc.sync.dma_start(x8_t[0][:, :, sq], x8_d[0][:, :, sq])
            nc.sync.dma_start(xr8_t[0][:, :, sq], xr8_d[0][:, :, sq])
        wo_r = load_c((128, NDT, C), F16, woutT_d, "wo", n_splits=NDT)
        bones_r = load_c((128, 128), F16, bones_d, "bones")
        cmem_r = load_c((128, NDT, 130), F16, cmem_d, "cmem")
        ident_r = load_c((128, 128), F16, ident_d, "ident")
        bq_t = const.tile([128, NDT], F32, tag="bq")
        nc.sync.dma_start(bq_t[:], bq_d)
        if has_bk or has_bv:
            onesrow_r = load_c((2, 128), F16, onesrow_d, "onesrow")
            bkv_r = load_c((2, 2 * MD), F16, bkv_d, "bkv")
        if has_bout:
            bout_t = const.tile([128, 2], F32, tag="bout")
            nc.sync.dma_start(bout_t[:], bout_d)
        for b in range(1, BPC):
            nc.sync.dma_start(x8_t[b][:], x8_d[b])
            nc.sync.dma_start(xr8_t[b][:], xr8_d[b])

        zero_r = const.tile([128, 128], F16, tag="zeror")
        nc.vector.memset(zero_r[:], 0.0)
        wzero = const.tile([128, 512], F16, tag="wzero")
        nc.vector.memset(wzero[:], 0.0)

        # persistent blockdiag buffers: off-diagonal zeros written once
        cds, cdts = [], []
        for p in range(NDT):
            cd = const.tile([128, 128], F16, tag=f"cd{p}", name=f"cd{p}")
            cdt = const.tile([128, 128], F16, tag=f"cdt{p}", name=f"cdt{p}")
            nc.vector.memset(cd[:], 0.0)
            nc.vector.memset(cdt[:], 0.0)
            cds.append(cd)
            cdts.append(cdt)
        rz4 = const.tile([128, NDT], F32, tag="rz4")
        # vt ring: ones cols (Z accumulators) written once, v-cols each s-tile
        vts = []
        for i in range(8):
            vt = const.tile([128, NDT, 130], F16, tag=f"vt{i}", name=f"vt{i}")
            nc.vector.memset(vt[:, :, 128:130], 1.0)
            vts.append(vt)

        # PE warmup: ramps p-state and covers the batch-0 input DMA
        with tc.tile_pool(name="warm", bufs=1, space="PSUM") as pw:
            wps = pw.tile([128, 512], F32, tag="warm")
            for i in range(2):
                nc.tensor.matmul(wps[:], zero_r[:], wzero[:],
                                 start=True, stop=True)

        def emit_tile(pp, bt, st):
            # one pass-A s-tile for batch bt: k + v matmuls, exp, vt evac
            x8b, xr8b = x8_t[bt], xr8_t[bt]
            sl = slice(st * 128, (st + 1) * 128)
            kps = pp.tile([128, 512], F32, tag=f"k{st % 2}",
                          name=f"k_{bt}_{st}")
            nc.tensor.matmul(kps[:], x8b[:, :, sl], w8k_r[:],
                             start=True, stop=(not has_bk), perf_mode=DR)
            if has_bk:
                nc.tensor.matmul(kps[:], onesrow_r[:], bkv_r[:, 0:MD],
                                 start=False, stop=True)
            # v at x64 scale via fp8 DoubleRow with residual compensation:
            # x8*wv8 + x8*wvr8 + xr8*wv8 (descaled through w_out on host)
            vps = pp.tile([128, 512], F32, tag=f"v{st % 2}",
                          name=f"v_{bt}_{st}")
            nc.tensor.matmul(vps[:], x8b[:, :, sl], wv8_r[:],
                             start=True, stop=False, perf_mode=DR)
            nc.tensor.matmul(vps[:], x8b[:, :, sl], wvr8_r[:],
                             start=False, stop=False, perf_mode=DR)
            nc.tensor.matmul(vps[:], xr8b[:, :, sl], wv8_r[:],
                             start=False, stop=(not has_bv), perf_mode=DR)
            if has_bv:
                nc.tensor.matmul(vps[:], onesrow_r[:], bkv_r[:, MD:2 * MD],
                                 start=False, stop=True)
            ek = work.tile([128, 512], F16, tag="ek", name=f"ek_{bt}_{st}")
            nc.scalar.activation(ek[:], kps[:], AF.Exp, scale=1.0 / WS)
            vt = vts[st % 8]
            # PSUM evac must stay off GPSIMD (no PSUM access on Pool)
            nc.vector.tensor_copy(
                vt[:, :, 0:128],
                vps[:].rearrange("p (g c) -> p g c", g=NDT),
            )
            return (ek, vt, st)

        NPRE = 3  # s-tiles of the next batch pre-staged before the y drain
        pre_staged = {}
        for b in range(BPC):
            x8_r, xr8_r = x8_t[b], xr8_t[b]
            with tc.tile_pool(name=f"pp{b}", bufs=1, space="PSUM") as pp:
                # one PSUM bank per ctx pair: interleaved accumulation groups
                # must NOT share a bank (HW accumulate state is per-bank)
                ctxps = [
                    pp.tile([128, 130], F32, tag=f"c{p}", name=f"ctx{p}_{b}")[:]
                    for p in range(NDT)
                ]

                # ---- pass A ----
                def ctx_mms(ek, vt, st):
                    for p in range(NDT):
                        nc.tensor.matmul(
                            ctxps[p][:],
                            ek[:, p * 128:(p + 1) * 128],
                            vt[:, p, :],
                            start=(st == 0),
                            stop=False,
                        )

                apend = list(pre_staged.pop(b, []))
                for st in range(len(apend), NST):
                    entry = emit_tile(pp, b, st)
                    while len(apend) > 1:
                        ctx_mms(*apend.pop(0))
                    apend.append(entry)
                for it in apend:
                    ctx_mms(*it)
                # ---- mem add + ctx evac, interleaved per pair ----
                for p in range(NDT):
                    nc.tensor.matmul(ctxps[p][:], ident_r[:], cmem_r[:, p, :],
                                     start=False, stop=True)
                    nc.vector.reciprocal_approx_fast(
                        rz4[:, p:p + 1], ctxps[p][:, 128:129]
                    )
                    nc.scalar.copy(cds[p][0:64, 0:64], ctxps[p][0:64, 0:64])
                    nc.scalar.copy(cds[p][64:128, 64:128],
                                   ctxps[p][64:128, 64:128])
                    for h in range(2):
                        o = 64 * h
                        for a in range(2):
                            for cc in range(2):
                                nc.vector.transpose(
                                    cdts[p][o + 32 * a:o + 32 * a + 32,
                                            o + 32 * cc:o + 32 * cc + 32],
                                    cds[p][o + 32 * cc:o + 32 * cc + 32,
                                           o + 32 * a:o + 32 * a + 32],
                                )
                weffs = []
                for p in range(NDT):
                    wraw = pp.tile([128, C], F32, tag=f"k{p % 2}",
                                   name=f"wraw{b}_{p}")
                    nc.tensor.matmul(wraw[:], cdts[p][:], wo_r[:, p, :],
                                     start=True, stop=True)
                    weff = weffp.tile([128, C], F16, tag="weff",
                                      name=f"weff_{b}_{p}")
                    nc.vector.tensor_scalar_mul(weff[:], wraw[:], rz4[:, p:p + 1])
                    weffs.append(weff)

                # ---- pass B ----
                def stage_y(sc, eps):
                    s0 = sc * 512
                    for ot in range(2):
                        yps = pp.tile([128, 512], F32, tag=f"c{ot}",
                                      name=f"y{b}_{sc}_{ot}")
                        for d in range(NDT):
                            nc.tensor.matmul(
                                yps[:],
                                weffs[d][:, ot * 128:(ot + 1) * 128],
                                eps[d][:],
                                start=(d == 0),
                                stop=(d == NDT - 1),
                            )
                        ysb = pool9.tile([128, 512], F16, tag="ysb")
                        if has_bout:
                            nc.scalar.activation(ysb[:], yps[:], AF.Identity,
                                                 bias=bout_t[:, ot:ot + 1])
                        else:
                            nc.scalar.activation(ysb[:], yps[:], AF.Identity)
                        nc.sync.dma_start(y_d[b, ot, :, s0:s0 + 512], ysb[:])

                pending = []
                depth = 3
                for sc in range(NSC):
                    # drain BEFORE this chunk's exps: ACT is in-order, so ysb
                    # must precede exps whose inputs aren't ready yet
                    if len(pending) > depth:
                        stage_y(*pending.pop(0))
                    s0 = sc * 512
                    eps = []
                    for d in range(NDT):
                        # 4-bank rotation decouples q matmuls from exp
                        # backlog; chunk 0 avoids k0/k1 so the wraw matmuls
                        # (same banks) run concurrently with the first exps
                        qtag = ("v0", "v1", "v0", "v1") if sc == 0 else \
                            ("v0", "v1", "k0", "k1")
                        qps = pp.tile([128, 512], F32, tag=qtag[d],
                                      name=f"q{b}_{sc}_{d}")
                        nc.tensor.matmul(qps[:], w8q_r[:, d],
                                         x8_r[:, :, s0:s0 + 512],
                                         start=True, stop=True,
                                         perf_mode=DR)
                        eu = pool9.tile([128, 512], F16, tag="eu")
                        nc.scalar.activation(eu[:], qps[:], AF.Exp,
                                             bias=bq_t[:, d:d + 1],
                                             scale=1.0 / WS)
                        zqb = pp.tile([128, 512], F32, tag=f"c{2 + d % 2}",
                                      name=f"zqb{b}_{sc}_{d}")
                        nc.tensor.matmul(zqb[:], bones_r[:], eu[:],
                                         start=True, stop=True)
                        rb = pool9.tile([128, 512], F32, tag="rb")
                        nc.vector.reciprocal_approx_fast(rb[:], zqb[:])
                        ep = pool9.tile([128, 512], F16, tag="ep")
                        # one normalize-mul on DVE, three on the idle GPSIMD
                        mul_eng = nc.vector if d == 0 else nc.gpsimd
                        mul_eng.tensor_mul(ep[:], eu[:], rb[:])
                        eps.append(ep)
                    pending.append((sc, eps))
                if b + 1 < BPC:
                    # pre-stage the next batch's first s-tiles before the y
                    # drain: k/v banks are free, and this feeds DVE/ACT work
                    # (vt, exp) into the drain where they would idle
                    pre_staged[b + 1] = [emit_tile(pp, b + 1, st)
                                        for st in range(NPRE)]
                for it in pending:
                    stage_y(*it)
        cstack.close()

    nc.compile()
    return nc


def _prep_consts(w_qkv, b_qkv, mem_kv, w_out, b_out, bn_gamma, bn_beta, bn_mean, bn_var):
    w_qkv = np.asarray(w_qkv, np.float32)
    b_qkv = np.asarray(b_qkv, np.float32)
    mem_kv = np.asarray(mem_kv, np.float32)
    w_out = np.asarray(w_out, np.float32)
    b_out = np.asarray(b_out, np.float32)
    g = np.asarray(bn_gamma, np.float64)
    be = np.asarray(bn_beta, np.float64)
    mu = np.asarray(bn_mean, np.float64)
    var = np.asarray(bn_var, np.float64)

    inv = g / np.sqrt(var + EPS)
    # SCALE (softmax(q) * HD**-0.5) is folded into the output projection
    w_out_f = (w_out.astype(np.float64) * inv[:, None] * SCALE).astype(np.float32)
    b_out_f = ((b_out.astype(np.float64) - mu) * inv + be).astype(np.float32)

    wq = w_qkv[0:MD]          # [512, 256]
    wk = w_qkv[MD:2 * MD]
    wv = w_qkv[2 * MD:]

    consts = {}
    # w8k[p, t, o] = wk[o, t*128+p] * WS
    consts["w8k"] = np.ascontiguousarray(
        (wk.T * WS).reshape(2, 128, MD).transpose(1, 0, 2)
    ).astype(NP8)
    # w8q[p, d, t, m] = wq[d*128+m, t*128+p] * WS
    w8q = (wq * WS).reshape(NDT, 128, 2, 128)  # [d, m, t, p]
    consts["w8q"] = np.ascontiguousarray(w8q.transpose(3, 0, 2, 1)).astype(NP8)
    # v weights at x64 scale + fp8 residual; the x64 is descaled via w_out
    wv64 = np.ascontiguousarray((wv.T * WS).reshape(2, 128, MD).transpose(1, 0, 2))
    wv8 = wv64.astype(NP8)
    consts["wv8"] = wv8
    consts["wvr8"] = (wv64 - wv8.astype(np.float32)).astype(NP8)
    consts["woutT"] = np.ascontiguousarray(
        (w_out_f / WS).T.reshape(NDT, 128, C)
    ).astype(np.float16)
    consts["bq"] = np.ascontiguousarray(b_qkv[0:MD].reshape(NDT, 128).T)

    bones = np.zeros((128, 128), np.float16)
    bones[0:64, 0:64] = 1.0
    bones[64:128, 64:128] = 1.0
    consts["bones"] = bones

    mk = mem_kv[0].astype(np.float64)
    mv = mem_kv[1].astype(np.float64)
    emk = np.exp(mk)
    ctx_mem = np.einsum("him,hjm->hij", emk, mv)
    z_mem = emk.sum(-1)
    cmem = np.zeros((128, NDT, 130), np.float16)
    for p in range(NDT):
        for t in range(2):
            h = 2 * p + t
            r0 = 64 * t
            # ctx rows are at the x64 v-scale; Z col stays unscaled
            cmem[r0:r0 + 64, p, r0:r0 + 64] = ctx_mem[h] * WS
            cmem[r0:r0 + 64, p, 128] = z_mem[h]
    consts["cmem"] = cmem
    consts["ident"] = np.eye(128, dtype=np.float16)

    has_bk = bool(np.any(b_qkv[MD:2 * MD] != 0))
    has_bv = bool(np.any(b_qkv[2 * MD:] != 0))
    has_bout = bool(np.any(b_out_f != 0))
    if has_bk or has_bv:
        # K=2 rank-2 form: ones row + zero row (k bias pre-scaled like w8k)
        onesrow = np.zeros((2, 128), np.float16)
        onesrow[0] = 1.0
        consts["onesrow"] = onesrow
        bkv = np.zeros((2, 2 * MD), np.float16)
        bkv[0, 0:MD] = (b_qkv[MD:2 * MD] * WS).astype(np.float16)
        bkv[0, MD:] = (b_qkv[2 * MD:] * WS).astype(np.float16)
        consts["bkv"] = bkv
    if has_bout:
        consts["bout"] = np.ascontiguousarray(b_out_f.reshape(2, 128).T)
    return consts, has_bk, has_bv, has_bout


def kernel(x, w_qkv, b_qkv, mem_kv, w_out, b_out, bn_gamma, bn_beta, bn_mean, bn_var):
    x = np.asarray(x, np.float32)
    consts, has_bk, has_bv, has_bout = _prep_consts(
        w_qkv, b_qkv, mem_kv, w_out, b_out, bn_gamma, bn_beta, bn_mean, bn_var
    )

    key = (has_bk, has_bv, has_bout)
    if key not in _MODULE_CACHE:
        _MODULE_CACHE[key] = _build_module(*key)
    nc = _MODULE_CACHE[key]

    # x8[b, p, t, s] = x[b, t*128+p, s]; xr8 = fp8 residual of the fp8 cast
    xt = x.reshape(B, 2, 128, S).transpose(0, 2, 1, 3)
    x8 = np.ascontiguousarray(xt).astype(NP8)
    xr8 = (xt - x8.astype(np.float32)).astype(NP8)
    in_maps = []
    for c in range(N_CORES):
        m = dict(consts)
        m["x8"] = np.ascontiguousarray(x8[c * BPC:(c + 1) * BPC])
        m["xr8"] = np.ascontiguousarray(xr8[c * BPC:(c + 1) * BPC])
        in_maps.append(m)

    trace = bool(int(os.environ.get("BASS_KERNEL_TRACE", "0")))
    res = bass_utils.run_bass_kernel_spmd(
        nc, in_maps, core_ids=list(range(N_CORES)), trace=trace
    )
    if trace:
        kernel.last_exec_time_ns = res.exec_time_ns
        kernel.last_mean_exec_time_ns = res.mean_exec_time_ns

    y = np.stack([res.results[c]["y"] for c in range(N_CORES)])
    y = y.reshape(B, C, H, W).astype(np.float32)
    return y
